# revision 21
# baseline (speedup 1.0000x reference)
"""Causal self-attention (T=4096, C=2048, 16 heads) on 8 TRN2 NeuronCores.

Sharding: tensor-parallel over heads (2 heads/core) for QKV + attention,
then one per-head AllToAll redistributes the (already-normalized)
attention output to token-parallel (512 tokens/core) for the output
projection. No reduction collective is needed: each core computes full
output rows for its token slice and the host concatenates.

All matmuls run bf16 (inputs converted to bf16 on the host, halving DMA
bytes; PSUM accumulation stays fp32). Scores are computed transposed
(keys on partitions, queries free) so P@V needs no transposes; causal
masking is a bf16 multiply with 4 precomputed diagonal mask tiles and
upper-triangle blocks are skipped entirely.

Key structure (evolved from a denominator-matmul-per-score-tile
baseline through trace analysis):
- softmax denominators come from a vector-engine running sum of the exp
  tiles (esum) + 2 small matmuls per chunk, instead of a ones-vector
  matmul per score tile (which cost a full 512-row PE pass each).
- exp activations cover two score tiles at once ([128,1024] across two
  PSUM banks) to amortize the scalar-engine per-instruction overhead.
- softmax normalization happens on the SEND side of the A2A: per chunk,
  reciprocal_approx_fast on the [1,512] denominators -> bf16 ->
  partition_broadcast -> gpsimd multiply into the staged A2A tile.
  This removes the denominator A2As and the entire receive-side
  normalize chain, so phase 3 starts ~1us after the A2A lands.
- weights and x load as few large DMAs (host pre-packs two 128-row
  tiles per SBUF tile) split across the scalar/sync HWDGE queues;
  w_proj prefetches during phase 1.
- phase 3 runs all head-0-sourced (even kc) matmuls first, parking
  even-kc partials in SBUF so the 8 PSUM banks can be reused: ~35us of
  PE work covers head 1's A2A latency; odd kc then chases the arriving
  shards kc-outer, with per-group add+store to stagger the tail.
"""
import sys
import types

sys.path.insert(0, "/opt/trn_rl_repo")

import ml_dtypes
import numpy as np

from concourse import bacc, tile
import concourse.mybir as mybir
from concourse.bass_utils import run_bass_kernel_spmd

F32 = mybir.dt.float32
BF16 = mybir.dt.bfloat16
NP_BF16 = np.dtype(ml_dtypes.bfloat16)

T, C = 4096, 2048
H, D = 16, 128
W = 8                  # cores
HL = H // W            # heads per core (2)
CL = HL * D            # local attention-output columns (256)
KT = C // 128          # contraction tiles (16)
TC1 = 512              # phase-1 token chunk
NC1 = T // TC1         # 8
TC2 = 512              # phase-2/3 token chunk
NC2 = T // TC2         # 8
TL = T // W            # tokens per core for the projection (512)
SCALE = float(1.0 / np.sqrt(D))

TRACE = False          # test harness sets kernel.TRACE = True for profiling
LAST_RESULT = {}       # test harness reads exec_time_ns from here

_cache = {}


def _build():
    nc = bacc.Bacc("TRN2", target_bir_lowering=False, debug=False, num_devices=W)
    # host packs pairs of 128-row k-tiles side by side so every input
    # loads as few large DMAs (tile r holds k-tiles 2r and 2r+1)
    xT_d = nc.dram_tensor("xT2", [C // 2, 2 * T], BF16, kind="ExternalInput")
    wqkvT_d = nc.dram_tensor("wqkvT2", [C // 2, 6 * CL], BF16,
                             kind="ExternalInput")
    wpT_d = nc.dram_tensor("wpT2", [C // 2, 2 * C], BF16, kind="ExternalInput")
    out_d = nc.dram_tensor("out", [TL, C], F32, kind="ExternalOutput")

    with tile.TileContext(nc) as tc:
        with tc.tile_pool(name="res", bufs=1) as res, \
             tc.tile_pool(name="dram", bufs=1, space="DRAM") as dram:
            # per-head A2A buffers (bf16): shard j = my token chunk j,
            # already softmax-normalized on the send side.
            a2a_in = [dram.tile([W, 128, TC2], BF16, tag=f"a2a_in{h}",
                                name=f"a2a_in{h}") for h in range(HL)]
            a2a_out = [dram.tile([W, 128, TC2], BF16, tag=f"a2a_out{h}",
                                 name=f"a2a_out{h}") for h in range(HL)]

            # resident q/k (transposed, [d, t]) and V ([s, d]), all bf16
            qT = [res.tile([128, T], BF16, tag=f"qT{h}", name=f"qT{h}")
                  for h in range(HL)]
            kT = [res.tile([128, T], BF16, tag=f"kT{h}", name=f"kT{h}")
                  for h in range(HL)]
            V = [res.tile([128, CL], BF16, tag=f"V{i}", name=f"V{i}")
                 for i in range(T // 128)]

            ones32 = res.tile([128, 1], F32, tag="ones32")
            nc.gpsimd.memset(ones32[:], 1.0)
            ones = res.tile([128, 1], BF16, tag="ones")
            nc.vector.tensor_copy(ones[:], ones32[:])

            # 4 diagonal causal masks (keep where t >= s within the tile):
            # mask dk applies to s-tile k = 4j + dk of query chunk j
            masks = []
            m32 = res.tile([128, TC2], F32, tag="m32", name="m32")
            nc.gpsimd.memset(m32[:], 1.0)
            for dk in range(4):
                mb = res.tile([128, TC2], BF16, tag=f"mask{dk}",
                              name=f"mask{dk}")
                nc.vector.tensor_copy(mb[:], m32[:])
                nc.gpsimd.affine_select(
                    out=mb[:], in_=mb[:],
                    compare_op=mybir.AluOpType.is_ge,
                    fill=0.0,
                    base=-128 * dk,
                    channel_multiplier=-1,
                    pattern=[[1, TC2]],
                )
                masks.append(mb)

            wp = []   # projection weight: 8 tiles [128, 4096], kc pair each

            # ---------------- phase 1: QKV projection (bf16) ----------------
            with tc.tile_pool(name="wpool", bufs=1) as wpool, \
                 tc.tile_pool(name="xpool", bufs=2) as xpool, \
                 tc.tile_pool(name="ps1q", bufs=3, space="PSUM") as ps1q, \
                 tc.tile_pool(name="ps1v", bufs=3, space="PSUM") as ps1v:
                # qkv weights: 8 big DMAs on the scalar HWDGE queue
                wqkv = []
                for r in range(KT // 2):
                    t_ = wpool.tile([128, 6 * CL], BF16, tag=f"wqkv{r}",
                                    name=f"wqkv{r}")
                    nc.scalar.dma_start(
                        t_[:], wqkvT_d.ap()[r * 128:(r + 1) * 128, :])
                    wqkv.append(t_)

                def wq_ap(k, lo, hi):
                    base = (k % 2) * 3 * CL
                    return wqkv[k // 2][:, base + lo:base + hi]

                def load_x_chunk(j):
                    xt = []
                    for r in range(KT // 2):
                        t_ = xpool.tile([128, 2 * TC1], BF16, tag=f"x{r}",
                                        name=f"x{j}_{r}")
                        nc.sync.dma_start(
                            t_[:],
                            xT_d.ap()[r * 128:(r + 1) * 128,
                                      j * 2 * TC1:(j + 1) * 2 * TC1],
                        )
                        xt.append(t_)
                    return xt

                def x_ap(xt, k, lo, hi):
                    base = (k % 2) * TC1
                    return xt[k // 2][:, base + lo:base + hi]

                xt0 = load_x_chunk(0)
                # prefetch w_proj now: 8 x 1MiB DMAs on the scalar queue,
                # they trickle in behind wqkv during phase-1 compute
                for r in range(KT // 2):
                    t_ = res.tile([128, 2 * C], BF16, tag=f"wp{r}",
                                  name=f"wp{r}")
                    nc.scalar.dma_start(
                        t_[:], wpT_d.ap()[r * 128:(r + 1) * 128, :])
                    wp.append(t_)

                def wp_ap(kc, lo, hi):
                    base = (kc % 2) * C
                    return wp[kc // 2][:, base + lo:base + hi]

                for j in range(NC1):
                    xt = xt0 if j == 0 else load_x_chunk(j)
                    # qT/kT for both heads: out[d, t] accumulated over c
                    for m in range(4):
                        pq = ps1q.tile([128, TC1], F32, tag="pqk")
                        for k in range(KT):
                            nc.tensor.matmul(
                                pq[:],
                                wq_ap(k, m * 128, (m + 1) * 128),
                                x_ap(xt, k, 0, TC1),
                                start=(k == 0), stop=(k == KT - 1))
                        dest = qT[m] if m < HL else kT[m - HL]
                        nc.vector.tensor_copy(
                            dest[:, j * TC1:(j + 1) * TC1], pq[:])
                    # V: out[t, d] accumulated over c
                    for tt in range(TC1 // 128):
                        pv = ps1v.tile([128, CL], F32, tag="pv")
                        for k in range(KT):
                            nc.tensor.matmul(
                                pv[:],
                                x_ap(xt, k, tt * 128, (tt + 1) * 128),
                                wq_ap(k, 2 * CL, 3 * CL),
                                start=(k == 0), stop=(k == KT - 1))
                        nc.scalar.copy(V[j * (TC1 // 128) + tt][:], pv[:])

            # ---------------- phases 2+3 pools ----------------
            with tc.tile_pool(name="ph2", bufs=4) as p2, \
                 tc.tile_pool(name="esp", bufs=2) as esp, \
                 tc.tile_pool(name="a2s", bufs=3) as a2s, \
                 tc.tile_pool(name="rcp", bufs=2) as rcp, \
                 tc.tile_pool(name="p3a", bufs=1) as p3a, \
                 tc.tile_pool(name="p3o", bufs=4) as p3o:

                attn = [None] * KT        # [128ch, TL] tiles, kc = i*HL + h

                def phase2_head(h, mid_cb=None):
                    """scores+softmax+P@V for local head h; fires its A2A.

                    The A2A tile for chunk j is normalized before sending:
                    pd holds the softmax denominators; its reciprocal is
                    broadcast to 128 partitions and multiplied into the
                    staged attention tile on gpsimd (off every hot queue).
                    """
                    for j in range(NC2):
                        if j == 5 and mid_cb is not None:
                            mid_cb()
                        # diagonal pairs first so their exp+mask latency
                        # hides under the following dense score matmuls;
                        # each entry is the first k of a 2-s-tile pair
                        plist = [4 * j, 4 * j + 2] + list(range(0, 4 * j, 2))
                        po = ps2o.tile([128, TC2], F32, tag="po")
                        esum = esp.tile([128, 2 * TC2], BF16, tag="esum")
                        for pi, k0 in enumerate(plist):
                            ps = ps2s.tile([128, 2 * TC2], F32, tag="ps")
                            for half in range(2):
                                k = k0 + half
                                nc.tensor.matmul(
                                    ps[:, half * TC2:(half + 1) * TC2],
                                    kT[h][:, k * 128:(k + 1) * 128],
                                    qT[h][:, j * TC2:(j + 1) * TC2],
                                    start=True, stop=True)
                            e = p2.tile([128, 2 * TC2], BF16, tag="e")
                            nc.scalar.activation(
                                e[:], ps[:],
                                mybir.ActivationFunctionType.Exp,
                                scale=SCALE)
                            for half in range(2):
                                dk = k0 + half - 4 * j
                                if 0 <= dk < 4:
                                    # diagonal tile: zero out s > t entries
                                    nc.vector.tensor_mul(
                                        e[:, half * TC2:(half + 1) * TC2],
                                        e[:, half * TC2:(half + 1) * TC2],
                                        masks[dk][:])
                            if pi == 0:
                                nc.vector.tensor_copy(esum[:], e[:])
                            else:
                                nc.vector.tensor_add(esum[:], esum[:], e[:])
                            for half in range(2):
                                k = k0 + half
                                nc.tensor.matmul(
                                    po[:],
                                    V[k][:, h * 128:(h + 1) * 128],
                                    e[:, half * TC2:(half + 1) * TC2],
                                    start=(pi == 0 and half == 0),
                                    stop=(pi == len(plist) - 1 and half == 1))
                        pd = ps2d.tile([1, TC2], F32, tag="pd")
                        nc.tensor.matmul(pd[:], ones[:], esum[:, 0:TC2],
                                         start=True, stop=False)
                        nc.tensor.matmul(pd[:], ones[:], esum[:, TC2:2 * TC2],
                                         start=False, stop=True)
                        # send-side softmax normalization
                        rec = rcp.tile([1, TC2], F32, tag="rec")
                        nc.vector.reciprocal_approx_fast(rec[:], pd[:])
                        recb = rcp.tile([1, TC2], BF16, tag="recb")
                        nc.vector.tensor_copy(recb[:], rec[:])
                        r128 = rcp.tile([128, TC2], BF16, tag="r128")
                        nc.gpsimd.partition_broadcast(r128[:], recb[:])
                        att = a2s.tile([128, TC2], BF16, tag="att")
                        nc.scalar.copy(att[:], po[:])
                        nc.gpsimd.tensor_mul(att[:], att[:], r128[:])
                        nc.sync.dma_start(a2a_in[h][j, :, :], att[:])
                    nc.gpsimd.collective_compute(
                        "AllToAll",
                        mybir.AluOpType.bypass,
                        ins=[a2a_in[h].opt()],
                        outs=[a2a_out[h].opt()],
                        replica_groups=[list(range(W))],
                    )

                def recv_head(h, engine):
                    """Load this head's A2A shards (already normalized)."""
                    for i in range(W):
                        kc = i * HL + h
                        t_ = p3a.tile([128, TL], BF16, tag=f"at{kc}",
                                      name=f"at{kc}")
                        engine.dma_start(t_[:], a2a_out[h][i, :, :])
                        attn[kc] = t_

                with tc.tile_pool(name="ps2s", bufs=2, space="PSUM") as ps2s, \
                     tc.tile_pool(name="ps2o", bufs=2, space="PSUM") as ps2o, \
                     tc.tile_pool(name="ps2d", bufs=2, space="PSUM") as ps2d:
                    phase2_head(0)
                    # head 0's shards load on the (otherwise idle) gpsimd
                    # SWDGE queue in the middle of head 1's compute, so a
                    # slow A2A can never block another engine's stream
                    phase2_head(1, mid_cb=lambda: recv_head(0, nc.gpsimd))
                    recv_head(1, nc.sync)

                # ---------------- phase 3: output projection (bf16) ----------
                # All 128 even-kc (head 0) matmuls run first — their shards
                # arrived mid-phase-2 — parking per-group partials in SBUF
                # so the 8 PSUM banks can be reused; this ~35us of PE work
                # covers head 1's A2A + load latency.  Odd kc then goes
                # kc-outer (chasing the arriving shards), and each group
                # finishes with partial+odd add -> store, staggered.
                with tc.tile_pool(name="ps3", bufs=1, space="PSUM") as ps3:
                    groups = {og: [(oc, tt) for oc in (2 * og, 2 * og + 1)
                                   for tt in range(TL // 128)]
                              for og in range(2)}
                    part = {}
                    for og in range(2):
                        for oc, tt in groups[og]:
                            po3 = ps3.tile([128, 512], F32,
                                           tag=f"po3_{oc % 2}_{tt}",
                                           name=f"po3e_{oc}_{tt}")
                            for kc in range(0, KT, 2):
                                nc.tensor.matmul(
                                    po3[:],
                                    attn[kc][:, tt * 128:(tt + 1) * 128],
                                    wp_ap(kc, oc * 512, (oc + 1) * 512),
                                    start=(kc == 0), stop=(kc == KT - 2))
                            pt = p3o.tile([128, 512], BF16,
                                          tag=f"pt{oc}_{tt}",
                                          name=f"pt{oc}_{tt}", bufs=1)
                            nc.scalar.copy(pt[:], po3[:])
                            part[(oc, tt)] = pt
                    for og in range(2):
                        po3s = {}
                        for oc, tt in groups[og]:
                            po3 = ps3.tile([128, 512], F32,
                                           tag=f"po3_{oc % 2}_{tt}",
                                           name=f"po3o_{oc}_{tt}")
                            po3s[(oc, tt)] = po3
                        for kc in range(1, KT - 2, 2):
                            for oc, tt in groups[og]:
                                nc.tensor.matmul(
                                    po3s[(oc, tt)][:],
                                    attn[kc][:, tt * 128:(tt + 1) * 128],
                                    wp_ap(kc, oc * 512, (oc + 1) * 512),
                                    start=(kc == 1), stop=False)
                        for oc, tt in groups[og]:
                            po3 = po3s[(oc, tt)]
                            nc.tensor.matmul(
                                po3[:],
                                attn[KT - 1][:, tt * 128:(tt + 1) * 128],
                                wp_ap(KT - 1, oc * 512, (oc + 1) * 512),
                                start=False, stop=True)
                            ob = p3o.tile([128, 512], F32, tag="ob")
                            nc.vector.tensor_add(ob[:], po3[:],
                                                 part[(oc, tt)][:])
                            nc.sync.dma_start(
                                out_d.ap()[tt * 128:(tt + 1) * 128,
                                           oc * 512:(oc + 1) * 512],
                                ob[:])

    nc.compile()
    return nc


def _maybe_install_trace_hook():
    try:
        import antenv
        from trn_agent_boot.trn_boot import _ntff_profile_via_ctypes
        hook = _ntff_profile_via_ctypes("/opt/axon/libaxon_pjrt.so")
        mod = types.ModuleType("antenv.axon_hooks")
        mod.get_axon_ntff_profile_hook = lambda: hook
        mod.set_axon_ntff_profile_hook = lambda h: None
        sys.modules["antenv.axon_hooks"] = mod
        antenv.axon_hooks = mod
        return True
    except Exception:
        return False


def _pack_pairs(a):
    """[2048, N] -> [1024, 2N]: tile r holds 128-row blocks 2r | 2r+1."""
    n = a.shape[1]
    return np.ascontiguousarray(
        a.reshape(KT // 2, 2, 128, n).transpose(0, 2, 1, 3).reshape(
            C // 2, 2 * n))


def kernel(x, w_attn, w_proj):
    x = np.ascontiguousarray(x, dtype=np.float32)
    w_attn = np.ascontiguousarray(w_attn, dtype=np.float32)
    w_proj = np.ascontiguousarray(w_proj, dtype=np.float32)

    if "nc" not in _cache:
        _cache["nc"] = _build()
    nc = _cache["nc"]

    # x pack: tile r, chunk j -> [k-tile 2r cols | k-tile 2r+1 cols]
    xT = np.ascontiguousarray(x.T).astype(NP_BF16)          # [C, T]
    xT2 = np.ascontiguousarray(
        xT.reshape(KT // 2, 2, 128, NC1, TC1)
        .transpose(0, 2, 3, 1, 4)
        .reshape(C // 2, 2 * T))
    wpT2 = _pack_pairs(np.ascontiguousarray(w_proj.T).astype(NP_BF16))
    in_maps = []
    for c in range(W):
        r0 = CL * c
        # columns: [q-heads | k-heads | v-heads] for this core, transposed
        wqkv = np.concatenate(
            [w_attn[r0:r0 + CL],
             w_attn[C + r0:C + r0 + CL],
             w_attn[2 * C + r0:2 * C + r0 + CL]], axis=0)
        wqkvT2 = _pack_pairs(np.ascontiguousarray(wqkv.T).astype(NP_BF16))
        in_maps.append({"xT2": xT2, "wqkvT2": wqkvT2, "wpT2": wpT2})

    trace = TRACE and _maybe_install_trace_hook()
    res = run_bass_kernel_spmd(nc, in_maps, list(range(W)), trace=trace)
    LAST_RESULT["exec_time_ns"] = res.exec_time_ns

    return np.concatenate([res.results[c]["out"] for c in range(W)], axis=0)


# revision 23
# speedup vs baseline: 1.0062x; 1.0062x over previous
"""Causal self-attention (T=4096, C=2048, 16 heads) on 8 TRN2 NeuronCores.

Sharding: tensor-parallel over heads (2 heads/core) for QKV + attention,
then one per-head AllToAll redistributes the (already-normalized)
attention output to token-parallel (512 tokens/core) for the output
projection. No reduction collective is needed: each core computes full
output rows for its token slice and the host concatenates.

All matmuls run bf16 (inputs converted to bf16 on the host, halving DMA
bytes; PSUM accumulation stays fp32). Scores are computed transposed
(keys on partitions, queries free) so P@V needs no transposes; causal
masking is a bf16 multiply with 4 precomputed diagonal mask tiles and
upper-triangle blocks are skipped entirely.

Key structure (evolved from a denominator-matmul-per-score-tile
baseline through trace analysis):
- softmax denominators come from a vector-engine running sum of the exp
  tiles (esum) + 2 small matmuls per chunk, instead of a ones-vector
  matmul per score tile (which cost a full 512-row PE pass each).
- exp activations cover two score tiles at once ([128,1024] across two
  PSUM banks) to amortize the scalar-engine per-instruction overhead.
- softmax normalization happens on the SEND side of the A2A: per chunk,
  reciprocal_approx_fast on the [1,512] denominators -> bf16 ->
  partition_broadcast -> gpsimd multiply into the staged A2A tile.
  This removes the denominator A2As and the entire receive-side
  normalize chain, so phase 3 starts ~1us after the A2A lands.
- weights and x load as few large DMAs (host pre-packs two 128-row
  tiles per SBUF tile) split across the scalar/sync HWDGE queues;
  w_proj prefetches during phase 1.
- phase 3 runs all head-0-sourced (even kc) matmuls first, parking
  even-kc partials in SBUF so the 8 PSUM banks can be reused: ~35us of
  PE work covers head 1's A2A latency; odd kc then chases the arriving
  shards kc-outer, with per-group add+store to stagger the tail.
"""
import sys
import types

sys.path.insert(0, "/opt/trn_rl_repo")

import ml_dtypes
import numpy as np

from concourse import bacc, tile
import concourse.mybir as mybir
from concourse.bass_utils import run_bass_kernel_spmd

F32 = mybir.dt.float32
BF16 = mybir.dt.bfloat16
NP_BF16 = np.dtype(ml_dtypes.bfloat16)

T, C = 4096, 2048
H, D = 16, 128
W = 8                  # cores
HL = H // W            # heads per core (2)
CL = HL * D            # local attention-output columns (256)
KT = C // 128          # contraction tiles (16)
TC1 = 512              # phase-1 token chunk
NC1 = T // TC1         # 8
TC2 = 512              # phase-2/3 token chunk
NC2 = T // TC2         # 8
TL = T // W            # tokens per core for the projection (512)
SCALE = float(1.0 / np.sqrt(D))

TRACE = False          # test harness sets kernel.TRACE = True for profiling
LAST_RESULT = {}       # test harness reads exec_time_ns from here

_cache = {}


def _build():
    nc = bacc.Bacc("TRN2", target_bir_lowering=False, debug=False, num_devices=W)
    # host packs pairs of 128-row k-tiles side by side so every input
    # loads as few large DMAs (tile r holds k-tiles 2r and 2r+1)
    xT_d = nc.dram_tensor("xT2", [C // 2, 2 * T], BF16, kind="ExternalInput")
    wqkvT_d = nc.dram_tensor("wqkvT2", [C // 2, 6 * CL], BF16,
                             kind="ExternalInput")
    wpT_d = nc.dram_tensor("wpT2", [C // 2, 2 * C], BF16, kind="ExternalInput")
    out_d = nc.dram_tensor("out", [TL, C], F32, kind="ExternalOutput")

    with tile.TileContext(nc) as tc:
        with tc.tile_pool(name="res", bufs=1) as res, \
             tc.tile_pool(name="dram", bufs=1, space="DRAM") as dram:
            # per-head A2A buffers (bf16): shard j = my token chunk j,
            # already softmax-normalized on the send side.
            a2a_in = [dram.tile([W, 128, TC2], BF16, tag=f"a2a_in{h}",
                                name=f"a2a_in{h}") for h in range(HL)]
            a2a_out = [dram.tile([W, 128, TC2], BF16, tag=f"a2a_out{h}",
                                 name=f"a2a_out{h}") for h in range(HL)]

            # resident q/k (transposed, [d, t]) and V ([s, d]), all bf16
            qT = [res.tile([128, T], BF16, tag=f"qT{h}", name=f"qT{h}")
                  for h in range(HL)]
            kT = [res.tile([128, T], BF16, tag=f"kT{h}", name=f"kT{h}")
                  for h in range(HL)]
            V = [res.tile([128, CL], BF16, tag=f"V{i}", name=f"V{i}")
                 for i in range(T // 128)]

            ones32 = res.tile([128, 1], F32, tag="ones32")
            nc.gpsimd.memset(ones32[:], 1.0)
            ones = res.tile([128, 1], BF16, tag="ones")
            nc.vector.tensor_copy(ones[:], ones32[:])

            # 4 diagonal causal masks (keep where t >= s within the tile):
            # mask dk applies to s-tile k = 4j + dk of query chunk j
            masks = []
            m32 = res.tile([128, TC2], F32, tag="m32", name="m32")
            nc.gpsimd.memset(m32[:], 1.0)
            for dk in range(4):
                mb = res.tile([128, TC2], BF16, tag=f"mask{dk}",
                              name=f"mask{dk}")
                nc.vector.tensor_copy(mb[:], m32[:])
                nc.gpsimd.affine_select(
                    out=mb[:], in_=mb[:],
                    compare_op=mybir.AluOpType.is_ge,
                    fill=0.0,
                    base=-128 * dk,
                    channel_multiplier=-1,
                    pattern=[[1, TC2]],
                )
                masks.append(mb)

            wp = []   # projection weight: 8 tiles [128, 4096], kc pair each

            # ---------------- phase 1: QKV projection (bf16) ----------------
            with tc.tile_pool(name="wpool", bufs=1) as wpool, \
                 tc.tile_pool(name="xpool", bufs=2) as xpool, \
                 tc.tile_pool(name="ps1q", bufs=3, space="PSUM") as ps1q, \
                 tc.tile_pool(name="ps1v", bufs=3, space="PSUM") as ps1v:
                # qkv weights: 8 big DMAs on the scalar HWDGE queue
                wqkv = []
                for r in range(KT // 2):
                    t_ = wpool.tile([128, 6 * CL], BF16, tag=f"wqkv{r}",
                                    name=f"wqkv{r}")
                    nc.scalar.dma_start(
                        t_[:], wqkvT_d.ap()[r * 128:(r + 1) * 128, :])
                    wqkv.append(t_)

                def wq_ap(k, lo, hi):
                    base = (k % 2) * 3 * CL
                    return wqkv[k // 2][:, base + lo:base + hi]

                def load_x_chunk(j):
                    xt = []
                    for r in range(KT // 2):
                        t_ = xpool.tile([128, 2 * TC1], BF16, tag=f"x{r}",
                                        name=f"x{j}_{r}")
                        nc.sync.dma_start(
                            t_[:],
                            xT_d.ap()[r * 128:(r + 1) * 128,
                                      j * 2 * TC1:(j + 1) * 2 * TC1],
                        )
                        xt.append(t_)
                    return xt

                def x_ap(xt, k, lo, hi):
                    base = (k % 2) * TC1
                    return xt[k // 2][:, base + lo:base + hi]

                xt0 = load_x_chunk(0)
                # prefetch w_proj now: 8 x 1MiB DMAs on the scalar queue,
                # they trickle in behind wqkv during phase-1 compute
                for r in range(KT // 2):
                    t_ = res.tile([128, 2 * C], BF16, tag=f"wp{r}",
                                  name=f"wp{r}")
                    nc.scalar.dma_start(
                        t_[:], wpT_d.ap()[r * 128:(r + 1) * 128, :])
                    wp.append(t_)

                def wp_ap(kc, lo, hi):
                    base = (kc % 2) * C
                    return wp[kc // 2][:, base + lo:base + hi]

                for j in range(NC1):
                    xt = xt0 if j == 0 else load_x_chunk(j)
                    # qT/kT for both heads: out[d, t] accumulated over c
                    for m in range(4):
                        pq = ps1q.tile([128, TC1], F32, tag="pqk")
                        for k in range(KT):
                            nc.tensor.matmul(
                                pq[:],
                                wq_ap(k, m * 128, (m + 1) * 128),
                                x_ap(xt, k, 0, TC1),
                                start=(k == 0), stop=(k == KT - 1))
                        dest = qT[m] if m < HL else kT[m - HL]
                        nc.vector.tensor_copy(
                            dest[:, j * TC1:(j + 1) * TC1], pq[:])
                    # V: out[t, d] accumulated over c
                    for tt in range(TC1 // 128):
                        pv = ps1v.tile([128, CL], F32, tag="pv")
                        for k in range(KT):
                            nc.tensor.matmul(
                                pv[:],
                                x_ap(xt, k, tt * 128, (tt + 1) * 128),
                                wq_ap(k, 2 * CL, 3 * CL),
                                start=(k == 0), stop=(k == KT - 1))
                        nc.scalar.copy(V[j * (TC1 // 128) + tt][:], pv[:])

            # ---------------- phases 2+3 pools ----------------
            with tc.tile_pool(name="ph2", bufs=4) as p2, \
                 tc.tile_pool(name="esp", bufs=2) as esp, \
                 tc.tile_pool(name="a2s", bufs=3) as a2s, \
                 tc.tile_pool(name="rcp", bufs=2) as rcp, \
                 tc.tile_pool(name="p3a", bufs=1) as p3a, \
                 tc.tile_pool(name="p3o", bufs=4) as p3o:

                attn = [None] * KT        # [128ch, TL] tiles, kc = i*HL + h

                def phase2_head(h, mid_cb=None):
                    """scores+softmax+P@V for local head h; fires its A2A.

                    The A2A tile for chunk j is normalized before sending:
                    pd holds the softmax denominators; its reciprocal is
                    broadcast to 128 partitions and multiplied into the
                    staged attention tile on gpsimd (off every hot queue).
                    """
                    for j in range(NC2):
                        if j == 5 and mid_cb is not None:
                            mid_cb()
                        # diagonal pairs first so their exp+mask latency
                        # hides under the following dense score matmuls;
                        # each entry is the first k of a 2-s-tile pair
                        plist = [4 * j, 4 * j + 2] + list(range(0, 4 * j, 2))
                        po = ps2o.tile([128, TC2], F32, tag="po")
                        esum = esp.tile([128, 2 * TC2], BF16, tag="esum")
                        for pi, k0 in enumerate(plist):
                            ps = ps2s.tile([128, 2 * TC2], F32, tag="ps")
                            for half in range(2):
                                k = k0 + half
                                nc.tensor.matmul(
                                    ps[:, half * TC2:(half + 1) * TC2],
                                    kT[h][:, k * 128:(k + 1) * 128],
                                    qT[h][:, j * TC2:(j + 1) * TC2],
                                    start=True, stop=True)
                            e = p2.tile([128, 2 * TC2], BF16, tag="e")
                            nc.scalar.activation(
                                e[:], ps[:],
                                mybir.ActivationFunctionType.Exp,
                                scale=SCALE)
                            for half in range(2):
                                dk = k0 + half - 4 * j
                                if 0 <= dk < 4:
                                    # diagonal tile: zero out s > t entries
                                    nc.vector.tensor_mul(
                                        e[:, half * TC2:(half + 1) * TC2],
                                        e[:, half * TC2:(half + 1) * TC2],
                                        masks[dk][:])
                            if pi == 0:
                                nc.vector.tensor_copy(esum[:], e[:])
                            else:
                                nc.vector.tensor_add(esum[:], esum[:], e[:])
                            for half in range(2):
                                k = k0 + half
                                nc.tensor.matmul(
                                    po[:],
                                    V[k][:, h * 128:(h + 1) * 128],
                                    e[:, half * TC2:(half + 1) * TC2],
                                    start=(pi == 0 and half == 0),
                                    stop=(pi == len(plist) - 1 and half == 1))
                        pd = ps2d.tile([1, TC2], F32, tag="pd")
                        nc.tensor.matmul(pd[:], ones[:], esum[:, 0:TC2],
                                         start=True, stop=False)
                        nc.tensor.matmul(pd[:], ones[:], esum[:, TC2:2 * TC2],
                                         start=False, stop=True)
                        # send-side softmax normalization
                        rec = rcp.tile([1, TC2], F32, tag="rec")
                        nc.vector.reciprocal_approx_fast(rec[:], pd[:])
                        recb = rcp.tile([1, TC2], BF16, tag="recb")
                        nc.vector.tensor_copy(recb[:], rec[:])
                        r128 = rcp.tile([128, TC2], BF16, tag="r128")
                        nc.gpsimd.partition_broadcast(r128[:], recb[:])
                        att = a2s.tile([128, TC2], BF16, tag="att")
                        nc.vector.tensor_copy(att[:], po[:])
                        nc.gpsimd.tensor_mul(att[:], att[:], r128[:])
                        nc.sync.dma_start(a2a_in[h][j, :, :], att[:])
                    nc.gpsimd.collective_compute(
                        "AllToAll",
                        mybir.AluOpType.bypass,
                        ins=[a2a_in[h].opt()],
                        outs=[a2a_out[h].opt()],
                        replica_groups=[list(range(W))],
                    )

                def recv_head(h, engine):
                    """Load this head's A2A shards (already normalized)."""
                    for i in range(W):
                        kc = i * HL + h
                        t_ = p3a.tile([128, TL], BF16, tag=f"at{kc}",
                                      name=f"at{kc}")
                        engine.dma_start(t_[:], a2a_out[h][i, :, :])
                        attn[kc] = t_

                with tc.tile_pool(name="ps2s", bufs=2, space="PSUM") as ps2s, \
                     tc.tile_pool(name="ps2o", bufs=2, space="PSUM") as ps2o, \
                     tc.tile_pool(name="ps2d", bufs=2, space="PSUM") as ps2d:
                    phase2_head(0)
                    phase2_head(1)
                    # head 0's shards landed mid-phase-2; loading them here
                    # (sync queue, after every phase-2 write) never blocks a
                    # busy stream, and phase 3's even-kc matmuls only need
                    # the first shards ~1us after these issue
                    recv_head(0, nc.sync)
                    recv_head(1, nc.sync)

                # ---------------- phase 3: output projection (bf16) ----------
                # All 128 even-kc (head 0) matmuls run first — their shards
                # arrived mid-phase-2 — parking per-group partials in SBUF
                # so the 8 PSUM banks can be reused; this ~35us of PE work
                # covers head 1's A2A + load latency.  Odd kc then goes
                # kc-outer (chasing the arriving shards), and each group
                # finishes with partial+odd add -> store, staggered.
                with tc.tile_pool(name="ps3", bufs=1, space="PSUM") as ps3:
                    groups = {og: [(oc, tt) for oc in (2 * og, 2 * og + 1)
                                   for tt in range(TL // 128)]
                              for og in range(2)}
                    part = {}
                    for og in range(2):
                        for oc, tt in groups[og]:
                            po3 = ps3.tile([128, 512], F32,
                                           tag=f"po3_{oc % 2}_{tt}",
                                           name=f"po3e_{oc}_{tt}")
                            for kc in range(0, KT, 2):
                                nc.tensor.matmul(
                                    po3[:],
                                    attn[kc][:, tt * 128:(tt + 1) * 128],
                                    wp_ap(kc, oc * 512, (oc + 1) * 512),
                                    start=(kc == 0), stop=(kc == KT - 2))
                            pt = p3o.tile([128, 512], BF16,
                                          tag=f"pt{oc}_{tt}",
                                          name=f"pt{oc}_{tt}", bufs=1)
                            nc.scalar.copy(pt[:], po3[:])
                            part[(oc, tt)] = pt
                    for og in range(2):
                        po3s = {}
                        for oc, tt in groups[og]:
                            po3 = ps3.tile([128, 512], F32,
                                           tag=f"po3_{oc % 2}_{tt}",
                                           name=f"po3o_{oc}_{tt}")
                            po3s[(oc, tt)] = po3
                        for kc in range(1, KT - 2, 2):
                            for oc, tt in groups[og]:
                                nc.tensor.matmul(
                                    po3s[(oc, tt)][:],
                                    attn[kc][:, tt * 128:(tt + 1) * 128],
                                    wp_ap(kc, oc * 512, (oc + 1) * 512),
                                    start=(kc == 1), stop=False)
                        for oc, tt in groups[og]:
                            po3 = po3s[(oc, tt)]
                            nc.tensor.matmul(
                                po3[:],
                                attn[KT - 1][:, tt * 128:(tt + 1) * 128],
                                wp_ap(KT - 1, oc * 512, (oc + 1) * 512),
                                start=False, stop=True)
                            ob = p3o.tile([128, 512], F32, tag="ob")
                            nc.vector.tensor_add(ob[:], po3[:],
                                                 part[(oc, tt)][:])
                            nc.sync.dma_start(
                                out_d.ap()[tt * 128:(tt + 1) * 128,
                                           oc * 512:(oc + 1) * 512],
                                ob[:])

    nc.compile()
    return nc


def _maybe_install_trace_hook():
    try:
        import antenv
        from trn_agent_boot.trn_boot import _ntff_profile_via_ctypes
        hook = _ntff_profile_via_ctypes("/opt/axon/libaxon_pjrt.so")
        mod = types.ModuleType("antenv.axon_hooks")
        mod.get_axon_ntff_profile_hook = lambda: hook
        mod.set_axon_ntff_profile_hook = lambda h: None
        sys.modules["antenv.axon_hooks"] = mod
        antenv.axon_hooks = mod
        return True
    except Exception:
        return False


def _pack_pairs(a):
    """[2048, N] -> [1024, 2N]: tile r holds 128-row blocks 2r | 2r+1."""
    n = a.shape[1]
    return np.ascontiguousarray(
        a.reshape(KT // 2, 2, 128, n).transpose(0, 2, 1, 3).reshape(
            C // 2, 2 * n))


def kernel(x, w_attn, w_proj):
    x = np.ascontiguousarray(x, dtype=np.float32)
    w_attn = np.ascontiguousarray(w_attn, dtype=np.float32)
    w_proj = np.ascontiguousarray(w_proj, dtype=np.float32)

    if "nc" not in _cache:
        _cache["nc"] = _build()
    nc = _cache["nc"]

    # x pack: tile r, chunk j -> [k-tile 2r cols | k-tile 2r+1 cols]
    xT = np.ascontiguousarray(x.T).astype(NP_BF16)          # [C, T]
    xT2 = np.ascontiguousarray(
        xT.reshape(KT // 2, 2, 128, NC1, TC1)
        .transpose(0, 2, 3, 1, 4)
        .reshape(C // 2, 2 * T))
    wpT2 = _pack_pairs(np.ascontiguousarray(w_proj.T).astype(NP_BF16))
    in_maps = []
    for c in range(W):
        r0 = CL * c
        # columns: [q-heads | k-heads | v-heads] for this core, transposed
        wqkv = np.concatenate(
            [w_attn[r0:r0 + CL],
             w_attn[C + r0:C + r0 + CL],
             w_attn[2 * C + r0:2 * C + r0 + CL]], axis=0)
        wqkvT2 = _pack_pairs(np.ascontiguousarray(wqkv.T).astype(NP_BF16))
        in_maps.append({"xT2": xT2, "wqkvT2": wqkvT2, "wpT2": wpT2})

    trace = TRACE and _maybe_install_trace_hook()
    res = run_bass_kernel_spmd(nc, in_maps, list(range(W)), trace=trace)
    LAST_RESULT["exec_time_ns"] = res.exec_time_ns

    return np.concatenate([res.results[c]["out"] for c in range(W)], axis=0)
